# revision 32
# baseline (speedup 1.0000x reference)
"""Trainium2 Bass kernel for MetaBayesLinearParallel.

Math (per sample s):
    W[s]  = weight_mu + weight_sigma * eps_w[s]          # (OUT, IN)
    Bv[s] = bias_mu + bias_sigma * eps_b[s]              # (OUT,)
    out[s] = x[s] @ W[s].T + Bv[s]                       # (B, OUT)

Sharding over 8 cores: 2-way split of the samples axis x 4-way split of
OUT (minimizes per-core HBM traffic).

All inputs are pre-packed on the host into each core's exact SBUF
layout: i-major (transposed), bf16 except mu which ships as fp8-e3m4
scaled by 256 (absolute quantization error ~1e-4 of W) and is
up-converted to bf16 by the otherwise-idle ACT engine.  Each sample's
x and eps are interleaved per i-block so a single SWDGE DMA feeds both
(descriptor generation on the Pool engine is ~1us per DMA instruction
and would otherwise pace the stream).  The device kernel is pure
streaming:
  DMA (SWDGE, span-chunked)  ->  DVE in-place wt = eps*sig; wt += mu
  ->  PE matmul psum[o,b] += wt_chunk.T @ x_chunk (bf16, fp32 PSUM)
  ->  ACT psum->SBUF copy with per-partition bias add (bf16 out)
  ->  per-ot stores spread across the HWDGE/SWDGE queues.
No PE transposes, no separate bias matmuls.  The last sample's final
span is chunked finer and emitted ot-major to shorten the drain.
"""

from contextlib import ExitStack

import numpy as np

import concourse.bacc as bacc
import concourse.mybir as mybir
import concourse.tile as tile
from concourse.bass_utils import run_bass_kernel_spmd

P = 128
S, B, IN, OUT = 8, 256, 2048, 2048
SAMPLE_WAYS, OUT_WAYS = 2, 4
N_CORES = SAMPLE_WAYS * OUT_WAYS
S_PC = S // SAMPLE_WAYS          # 4 samples per core
O_PC = OUT // OUT_WAYS           # 512 out rows per core
IB = IN // P                     # 16 i-blocks of 128
OT = O_PC // P                   # 4 o-blocks of 128
ISP = 4                          # chunking (4 ib per span)
IB_SP = IB // ISP

MU_SCALE = 256.0                 # host premultiplier for fp8 mu

# image column layout (elements per partition)
SIG_LEN = IB * O_PC                            # 8192 (bf16)
BLK = B + O_PC                                 # 768: [x_ib | eps_ib]
SMP_LEN = IB * BLK                             # 12288 per sample
SMP_OFF = SIG_LEN                              # samples follow sigma
IMG_COLS = SIG_LEN + S_PC * SMP_LEN            # 57344

BF16 = mybir.dt.bfloat16
F32 = mybir.dt.float32
FP8 = mybir.dt.float8e3


def _eps_chunks(s):
    """(ib_lo, ib_hi) DMA/DVE chunks for sample s."""
    chunks = [(isp * IB_SP, (isp + 1) * IB_SP) for isp in range(ISP - 1)]
    if s == S_PC - 1:
        chunks += [(IB - 4, IB - 2), (IB - 2, IB)]
    else:
        chunks += [(IB - IB_SP, IB)]
    if s > 0:
        # finer chunks: halves the DVE arrival quantum in the eps-dense
        # phase (descriptor generation still keeps ahead of the stream)
        chunks = [(k * 2, (k + 1) * 2) for k in range(IB // 2)]
    return chunks


def build_core_program(repeat=1):
    """One NeuronCore's program; identical on all cores (SPMD over slices)."""
    nc = bacc.Bacc("TRN2")
    img_d = nc.declare_dram_parameter("img", [P, IMG_COLS], BF16, isOutput=False)
    mu8_d = nc.declare_dram_parameter("mu8", [P, SIG_LEN], FP8, isOutput=False)
    bias_d = nc.declare_dram_parameter("bias", [P, 3 * S_PC * OT], F32, isOutput=False)
    out_d = nc.declare_dram_parameter("out", [S_PC, O_PC, B], BF16, isOutput=True)

    with ExitStack() as ctx:
        tc = ctx.enter_context(tile.TileContext(nc))
        resident = ctx.enter_context(tc.tile_pool(name="resident", bufs=1))
        biasp = ctx.enter_context(tc.tile_pool(name="biasp", bufs=1))
        outp = ctx.enter_context(tc.tile_pool(name="outp", bufs=8))
        psp = ctx.enter_context(tc.tile_pool(name="psp", bufs=8, space="PSUM"))

        for rep in range(repeat):
            _kernel_body(nc, tc, resident, biasp, outp, psp,
                         img_d, mu8_d, bias_d, out_d, rep)

    nc.compile()
    return nc


def _kernel_body(nc, tc, resident, biasp, outp, psp,
                 img_d, mu8_d, bias_d, out_d, rep):
    sig = resident.tile([P, SIG_LEN], BF16, tag="sig", name=f"sig_{rep}")
    smp = resident.tile([P, S_PC, IB, BLK], BF16, tag="smp", name=f"smp_{rep}")
    mu8 = resident.tile([P, SIG_LEN], FP8, tag="mu8", name=f"mu8_{rep}")
    mu_bf = resident.tile([P, SIG_LEN], BF16, tag="mubf", name=f"mubf_{rep}")
    bias_sb = biasp.tile([P, 3 * S_PC * OT], F32, tag="bias", name=f"bias_{rep}")

    # bias first on the HWDGE queue (tiny; needed by s0's ACT copy)
    nc.sync.dma_start(out=bias_sb[:], in_=bias_d[:, :])

    # ---- global task order -----------------------------------------------
    # s1's first chunks interleave into the musig-rich s0 phase so the DVE
    # (the tail-binding engine) converts early idle into progress instead
    # of accumulating an end-of-stream backlog.  s0+s1 PSUM coexist (8
    # half-bank tiles); s2/s3 follow sequentially.
    tasks = []           # (s, ci, lo, hi)
    ch = {s: _eps_chunks(s) for s in range(S_PC)}
    for ci in range(ISP):
        tasks.append((0, ci) + ch[0][ci])
        tasks.append((1, 2 * ci) + ch[1][2 * ci])
        tasks.append((1, 2 * ci + 1) + ch[1][2 * ci + 1])
    for s in (2, 3):
        for ci, (lo, hi) in enumerate(ch[s]):
            tasks.append((s, ci) + (lo, hi))

    # ---- input DMA issue order (SWDGE FIFO) ------------------------------
    for ti, (s, ci, lo, hi) in enumerate(tasks):
        q = nc.sync if ti == 0 else nc.gpsimd
        if s == 0:
            a, b = ci * IB_SP * O_PC, (ci + 1) * IB_SP * O_PC
            q.dma_start(out=mu8[:, a:b], in_=mu8_d[:, a:b])
            q.dma_start(out=sig[:, a:b], in_=img_d[:, a:b])
        da = SMP_OFF + s * SMP_LEN + lo * BLK
        db = SMP_OFF + s * SMP_LEN + hi * BLK
        q.dma_start(
            out=smp[:, s, lo:hi, :],
            in_=img_d[:, da:db].rearrange("p (q c) -> p q c", c=BLK))

    # ---- bias vector: bv[p, s*OT+ot] = bmu + bsig * eps_b ----------------
    nso = S_PC * OT
    nc.vector.tensor_mul(bias_sb[:, 0:nso], bias_sb[:, 0:nso],
                         bias_sb[:, nso:2 * nso])
    nc.vector.tensor_add(bias_sb[:, 0:nso], bias_sb[:, 0:nso],
                         bias_sb[:, 2 * nso:3 * nso])

    # ---- pipeline in task order ------------------------------------------
    po = {}

    def mm(s, ib, ot):
        nc.tensor.matmul(
            po[s][ot][:],
            smp[:, s, ib, B + ot * P:B + (ot + 1) * P],
            smp[:, s, ib, 0:B],
            start=(ib == 0), stop=(ib == IB - 1))

    for s, ci, lo, hi in tasks:
        if s not in po:
            po[s] = [psp.tile([P, B], F32, tag="psum",
                              name=f"ps_{rep}_{s}_{ot}") for ot in range(OT)]
        a, b = lo * O_PC, hi * O_PC
        if s == 0:
            # up-convert this span of mu on the ACT engine (idle early)
            nc.scalar.mul(mu_bf[:, a:b], mu8[:, a:b], 1.0 / MU_SCALE)
        # wt chunk in place: eps *= sig ; eps += mu
        ev = smp[:, s, lo:hi, B:BLK]
        nc.vector.tensor_mul(
            ev, ev, sig[:, a:b].rearrange("p (q c) -> p q c", c=O_PC))
        nc.vector.tensor_add(
            ev, ev, mu_bf[:, a:b].rearrange("p (q c) -> p q c", c=O_PC))
        if ci + 1 < len(ch[s]):
            for ib in range(lo, hi):
                for ot in range(OT):
                    mm(s, ib, ot)
            continue
        # final chunk of sample s: ot-major so psum[ot] completes staggered
        # and the copies/stores overlap the remaining matmuls
        if s == S_PC - 1:
            pairs = [outp.tile([P, 2, B], BF16, tag="o_pair",
                               name=f"op_{rep}_{k}") for k in range(2)]
            for ot in range(OT):
                for ib in range(lo, hi):
                    mm(s, ib, ot)
                c = s * OT + ot
                dst = pairs[ot // 2][:, ot % 2, :]
                if ot % 2 == 1:
                    nc.vector.tensor_scalar_add(dst, po[s][ot][:],
                                                bias_sb[:, c:c + 1])
                else:
                    nc.scalar.add(dst, po[s][ot][:],
                                  add=bias_sb[:, c:c + 1])
                if ot % 2 == 1:
                    q = nc.sync if ot == 1 else nc.gpsimd
                    q.dma_start(
                        out=out_d[s, (ot - 1) * P:(ot + 1) * P, :]
                        .rearrange("(o p) b -> p o b", p=P),
                        in_=pairs[ot // 2][:])
        else:
            for ot in range(OT):
                for ib in range(lo, hi):
                    mm(s, ib, ot)
                c = s * OT + ot
                o_sb = outp.tile([P, B], BF16, tag="o_sb",
                                 name=f"o_{rep}_{s}_{ot}")
                nc.scalar.add(o_sb[:], po[s][ot][:],
                              add=bias_sb[:, c:c + 1])
                nc.sync.dma_start(
                    out=out_d[s, ot * P:(ot + 1) * P, :],
                    in_=o_sb[:])


_prog_cache = {}
_last_in_maps = None


def _get_program(key=None):
    # key is accepted for compatibility; there is a single program variant
    if "prog" not in _prog_cache:
        _prog_cache["prog"] = build_core_program()
    return _prog_cache["prog"]


def _pack_inputs(x, weight_mu, weight_sigma, bias_mu, bias_sigma, eps_w, eps_b):
    """Per-core packed SBUF images + fp8 mu + bias blocks (host-side layout
    and dtype staging only — no model arithmetic)."""
    bf = mybir.dt.np(BF16)
    f8 = mybir.dt.np(FP8)
    in_maps = []
    for c in range(N_CORES):
        sg, og = divmod(c, OUT_WAYS)
        s_lo, o_lo = sg * S_PC, og * O_PC
        img = np.empty((P, IMG_COLS), dtype=bf)

        # mu/sig: [o, i] -> [p, ib, o]
        def t_os(w):
            return (w[o_lo:o_lo + O_PC].T
                    .reshape(IB, P, O_PC).transpose(1, 0, 2))  # [p, ib, o]
        img[:, 0:SIG_LEN] = t_os(weight_sigma).reshape(P, -1).astype(bf)
        mu8 = (t_os(weight_mu).reshape(P, -1) * MU_SCALE).astype(f8)

        # x: [s, b, i] -> [p, s, ib, b];  eps: [s, o, i] -> [p, s, ib, o]
        xs = x[s_lo:s_lo + S_PC].astype(bf)
        xT = xs.transpose(0, 2, 1).reshape(S_PC, IB, P, B).transpose(2, 0, 1, 3)
        es = eps_w[s_lo:s_lo + S_PC, o_lo:o_lo + O_PC, :].astype(bf)
        eT = (es.transpose(0, 2, 1).reshape(S_PC, IB, P, O_PC)
              .transpose(2, 0, 1, 3))
        img[:, SMP_OFF:] = np.concatenate([xT, eT], axis=3).reshape(P, -1)

        # bias block [p, 3*S_PC*OT] f32: [epsb | bsig_rep | bmu_rep]
        nso = S_PC * OT
        bias = np.empty((P, 3 * nso), dtype=np.float32)
        eb = eps_b[s_lo:s_lo + S_PC, o_lo:o_lo + O_PC]       # [4, 512]
        bias[:, 0:nso] = eb.reshape(S_PC, OT, P).transpose(2, 0, 1).reshape(P, -1)
        bs = bias_sigma[o_lo:o_lo + O_PC].reshape(OT, P).T   # [p, ot]
        bm = bias_mu[o_lo:o_lo + O_PC].reshape(OT, P).T
        bias[:, nso:2 * nso] = np.tile(bs, (1, S_PC))
        bias[:, 2 * nso:3 * nso] = np.tile(bm, (1, S_PC))

        in_maps.append({"img": img, "mu8": mu8, "bias": bias})
    return in_maps


def kernel(x, weight_mu, weight_sigma, bias_mu, bias_sigma, eps_w, eps_b):
    global _last_in_maps
    x = np.ascontiguousarray(x, dtype=np.float32)
    weight_mu = np.ascontiguousarray(weight_mu, dtype=np.float32)
    weight_sigma = np.ascontiguousarray(weight_sigma, dtype=np.float32)
    bias_mu = np.ascontiguousarray(bias_mu, dtype=np.float32)
    bias_sigma = np.ascontiguousarray(bias_sigma, dtype=np.float32)
    eps_w = np.ascontiguousarray(eps_w, dtype=np.float32)
    eps_b = np.ascontiguousarray(eps_b, dtype=np.float32)

    nc = _get_program()
    in_maps = _pack_inputs(x, weight_mu, weight_sigma, bias_mu, bias_sigma,
                           eps_w, eps_b)
    _last_in_maps = in_maps
    res = run_bass_kernel_spmd(nc, in_maps, core_ids=list(range(N_CORES)))

    out = np.empty((S, B, OUT), dtype=np.float32)
    for c in range(N_CORES):
        sg, og = divmod(c, OUT_WAYS)
        oT = np.asarray(res.results[c]["out"], dtype=np.float32)  # [4, 512, 256]
        out[sg * S_PC:(sg + 1) * S_PC, :, og * O_PC:(og + 1) * O_PC] = \
            oT.transpose(0, 2, 1)
    return out


# revision 34
# speedup vs baseline: 1.1628x; 1.1628x over previous
"""Trainium2 Bass kernel for MetaBayesLinearParallel.

Math (per sample s):
    W[s]  = weight_mu + weight_sigma * eps_w[s]          # (OUT, IN)
    Bv[s] = bias_mu + bias_sigma * eps_b[s]              # (OUT,)
    out[s] = x[s] @ W[s].T + Bv[s]                       # (B, OUT)

Sharding over 8 cores: 2-way split of the samples axis x 4-way split of
OUT (minimizes per-core HBM traffic).

All inputs are pre-packed on the host into each core's exact SBUF
layout: i-major (transposed), bf16 except mu which ships as fp8-e3m4
scaled by 256 (absolute quantization error ~1e-4 of W) and is
up-converted to bf16 by the otherwise-idle ACT engine.  Each sample's
x and eps are interleaved per i-block so a single SWDGE DMA feeds both
(descriptor generation on the Pool engine is ~1us per DMA instruction
and would otherwise pace the stream).  The device kernel is pure
streaming:
  DMA (SWDGE, span-chunked)  ->  DVE in-place wt = eps*sig; wt += mu
  ->  PE matmul psum[o,b] += wt_chunk.T @ x_chunk (bf16, fp32 PSUM)
  ->  ACT psum->SBUF copy with per-partition bias add (bf16 out)
  ->  per-ot stores spread across the HWDGE/SWDGE queues.
No PE transposes, no separate bias matmuls.  The last sample's final
span is chunked finer and emitted ot-major to shorten the drain.
"""

from contextlib import ExitStack

import numpy as np

import concourse.bacc as bacc
import concourse.mybir as mybir
import concourse.tile as tile
from concourse.bass_utils import run_bass_kernel_spmd

P = 128
S, B, IN, OUT = 8, 256, 2048, 2048
SAMPLE_WAYS, OUT_WAYS = 2, 4
N_CORES = SAMPLE_WAYS * OUT_WAYS
S_PC = S // SAMPLE_WAYS          # 4 samples per core
O_PC = OUT // OUT_WAYS           # 512 out rows per core
IB = IN // P                     # 16 i-blocks of 128
OT = O_PC // P                   # 4 o-blocks of 128
ISP = 4                          # chunking (4 ib per span)
IB_SP = IB // ISP

MU_SCALE = 256.0                 # host premultiplier for fp8 mu

# image column layout (elements per partition)
SIG_LEN = IB * O_PC                            # 8192 (bf16)
BLK = B + O_PC                                 # 768: [x_ib | eps_ib]
SMP_LEN = IB * BLK                             # 12288 per sample
SMP_OFF = SIG_LEN                              # samples follow sigma
IMG_COLS = SIG_LEN + S_PC * SMP_LEN            # 57344

BF16 = mybir.dt.bfloat16
F32 = mybir.dt.float32
FP8 = mybir.dt.float8e3


def _eps_chunks(s):
    """(ib_lo, ib_hi) DMA/DVE chunks for sample s."""
    chunks = [(isp * IB_SP, (isp + 1) * IB_SP) for isp in range(ISP - 1)]
    if s == S_PC - 1:
        chunks += [(IB - 4, IB - 2), (IB - 2, IB)]
    else:
        chunks += [(IB - IB_SP, IB)]
    if s > 0:
        # finer chunks: halves the DVE arrival quantum in the eps-dense
        # phase (descriptor generation still keeps ahead of the stream)
        chunks = [(k * 2, (k + 1) * 2) for k in range(IB // 2)]
    return chunks


def build_core_program(repeat=1):
    """One NeuronCore's program; identical on all cores (SPMD over slices)."""
    nc = bacc.Bacc("TRN2")
    img_d = nc.declare_dram_parameter("img", [P, IMG_COLS], BF16, isOutput=False)
    mu8_d = nc.declare_dram_parameter("mu8", [P, SIG_LEN], FP8, isOutput=False)
    bias_d = nc.declare_dram_parameter("bias", [P, 3 * S_PC * OT], F32, isOutput=False)
    out_d = nc.declare_dram_parameter("out", [S_PC, O_PC, B], BF16, isOutput=True)

    with ExitStack() as ctx:
        tc = ctx.enter_context(tile.TileContext(nc))
        resident = ctx.enter_context(tc.tile_pool(name="resident", bufs=1))
        biasp = ctx.enter_context(tc.tile_pool(name="biasp", bufs=1))
        outp = ctx.enter_context(tc.tile_pool(name="outp", bufs=8))
        psp = ctx.enter_context(tc.tile_pool(name="psp", bufs=8, space="PSUM"))

        for rep in range(repeat):
            _kernel_body(nc, tc, resident, biasp, outp, psp,
                         img_d, mu8_d, bias_d, out_d, rep)

    nc.compile()
    return nc


def _kernel_body(nc, tc, resident, biasp, outp, psp,
                 img_d, mu8_d, bias_d, out_d, rep):
    sig = resident.tile([P, SIG_LEN], BF16, tag="sig", name=f"sig_{rep}")
    smp = resident.tile([P, S_PC, IB, BLK], BF16, tag="smp", name=f"smp_{rep}")
    mu8 = resident.tile([P, SIG_LEN], FP8, tag="mu8", name=f"mu8_{rep}")
    mu_bf = resident.tile([P, SIG_LEN], BF16, tag="mubf", name=f"mubf_{rep}")
    bias_sb = biasp.tile([P, 3 * S_PC * OT], F32, tag="bias", name=f"bias_{rep}")

    # bias first on the HWDGE queue (tiny; needed by s0's ACT copy)
    nc.sync.dma_start(out=bias_sb[:], in_=bias_d[:, :])

    # ---- global task order -----------------------------------------------
    # s1's first chunks interleave into the musig-rich s0 phase so the DVE
    # (the tail-binding engine) converts early idle into progress instead
    # of accumulating an end-of-stream backlog.  s0+s1 PSUM coexist (8
    # half-bank tiles); s2/s3 follow sequentially.
    tasks = []           # (s, ci, lo, hi)
    ch = {s: _eps_chunks(s) for s in range(S_PC)}
    for ci in range(ISP):
        tasks.append((0, ci) + ch[0][ci])
        tasks.append((1, 2 * ci) + ch[1][2 * ci])
        tasks.append((1, 2 * ci + 1) + ch[1][2 * ci + 1])
    for s in (2, 3):
        for ci, (lo, hi) in enumerate(ch[s]):
            tasks.append((s, ci) + (lo, hi))

    # ---- input DMA issue order (SWDGE FIFO) ------------------------------
    for ti, (s, ci, lo, hi) in enumerate(tasks):
        q = nc.sync if ti == 0 else nc.gpsimd
        if s == 0:
            a, b = ci * IB_SP * O_PC, (ci + 1) * IB_SP * O_PC
            q.dma_start(out=mu8[:, a:b], in_=mu8_d[:, a:b])
            q.dma_start(out=sig[:, a:b], in_=img_d[:, a:b])
        da = SMP_OFF + s * SMP_LEN + lo * BLK
        db = SMP_OFF + s * SMP_LEN + hi * BLK
        q.dma_start(
            out=smp[:, s, lo:hi, :],
            in_=img_d[:, da:db].rearrange("p (q c) -> p q c", c=BLK))

    # ---- bias vector: bv[p, s*OT+ot] = bmu + bsig * eps_b ----------------
    nso = S_PC * OT
    nc.vector.tensor_mul(bias_sb[:, 0:nso], bias_sb[:, 0:nso],
                         bias_sb[:, nso:2 * nso])
    nc.vector.tensor_add(bias_sb[:, 0:nso], bias_sb[:, 0:nso],
                         bias_sb[:, 2 * nso:3 * nso])

    # ---- pipeline in task order ------------------------------------------
    po = {}

    def mm(s, ib, ot):
        nc.tensor.matmul(
            po[s][ot][:],
            smp[:, s, ib, B + ot * P:B + (ot + 1) * P],
            smp[:, s, ib, 0:B],
            start=(ib == 0), stop=(ib == IB - 1))

    for s, ci, lo, hi in tasks:
        if s not in po:
            po[s] = [psp.tile([P, B], F32, tag="psum",
                              name=f"ps_{rep}_{s}_{ot}") for ot in range(OT)]
        a, b = lo * O_PC, hi * O_PC
        if s == 0:
            # up-convert this span of mu on the ACT engine (idle early)
            nc.scalar.mul(mu_bf[:, a:b], mu8[:, a:b], 1.0 / MU_SCALE)
        # wt chunk in place: eps *= sig ; eps += mu
        ev = smp[:, s, lo:hi, B:BLK]
        nc.vector.tensor_mul(
            ev, ev, sig[:, a:b].rearrange("p (q c) -> p q c", c=O_PC))
        nc.vector.tensor_add(
            ev, ev, mu_bf[:, a:b].rearrange("p (q c) -> p q c", c=O_PC))
        if ci + 1 < len(ch[s]):
            for ib in range(lo, hi):
                for ot in range(OT):
                    mm(s, ib, ot)
            continue
        # final chunk of sample s: ot-major so psum[ot] completes staggered
        # and the copies/stores overlap the remaining matmuls
        if s == S_PC - 1:
            pairs = [outp.tile([P, 2, B], BF16, tag="o_pair",
                               name=f"op_{rep}_{k}") for k in range(2)]
            for ot in range(OT):
                for ib in range(lo, hi):
                    mm(s, ib, ot)
                c = s * OT + ot
                dst = pairs[ot // 2][:, ot % 2, :]
                if ot % 2 == 1:
                    nc.vector.tensor_scalar_add(dst, po[s][ot][:],
                                                bias_sb[:, c:c + 1])
                else:
                    nc.scalar.add(dst, po[s][ot][:],
                                  add=bias_sb[:, c:c + 1])
                if ot % 2 == 1:
                    q = nc.sync if ot == 1 else nc.gpsimd
                    q.dma_start(
                        out=out_d[s, (ot - 1) * P:(ot + 1) * P, :]
                        .rearrange("(o p) b -> p o b", p=P),
                        in_=pairs[ot // 2][:])
        else:
            for ot in range(OT):
                for ib in range(lo, hi):
                    mm(s, ib, ot)
                c = s * OT + ot
                o_sb = outp.tile([P, B], BF16, tag="o_sb",
                                 name=f"o_{rep}_{s}_{ot}")
                nc.scalar.add(o_sb[:], po[s][ot][:],
                              add=bias_sb[:, c:c + 1])
                nc.sync.dma_start(
                    out=out_d[s, ot * P:(ot + 1) * P, :],
                    in_=o_sb[:])


_prog_cache = {}
_last_in_maps = None


def _get_program(key=None):
    # key is accepted for compatibility; there is a single program variant
    if "prog" not in _prog_cache:
        _prog_cache["prog"] = build_core_program()
    return _prog_cache["prog"]


def _pack_inputs(x, weight_mu, weight_sigma, bias_mu, bias_sigma, eps_w, eps_b):
    """Per-core packed SBUF images + fp8 mu + bias blocks (host-side layout
    and dtype staging only — no model arithmetic)."""
    bf = mybir.dt.np(BF16)
    f8 = mybir.dt.np(FP8)
    in_maps = []
    for c in range(N_CORES):
        sg, og = divmod(c, OUT_WAYS)
        s_lo, o_lo = sg * S_PC, og * O_PC
        img = np.empty((P, IMG_COLS), dtype=bf)

        # mu/sig: [o, i] -> [p, ib, o]
        def t_os(w):
            return (w[o_lo:o_lo + O_PC].T
                    .reshape(IB, P, O_PC).transpose(1, 0, 2))  # [p, ib, o]
        img[:, 0:SIG_LEN] = t_os(weight_sigma).reshape(P, -1).astype(bf)
        mu8 = (t_os(weight_mu).reshape(P, -1) * MU_SCALE).astype(f8)

        # x: [s, b, i] -> [p, s, ib, b];  eps: [s, o, i] -> [p, s, ib, o]
        xs = x[s_lo:s_lo + S_PC].astype(bf)
        xT = xs.transpose(0, 2, 1).reshape(S_PC, IB, P, B).transpose(2, 0, 1, 3)
        es = eps_w[s_lo:s_lo + S_PC, o_lo:o_lo + O_PC, :].astype(bf)
        eT = (es.transpose(0, 2, 1).reshape(S_PC, IB, P, O_PC)
              .transpose(2, 0, 1, 3))
        img[:, SMP_OFF:] = np.concatenate([xT, eT], axis=3).reshape(P, -1)

        # bias block [p, 3*S_PC*OT] f32: [epsb | bsig_rep | bmu_rep]
        nso = S_PC * OT
        bias = np.empty((P, 3 * nso), dtype=np.float32)
        eb = eps_b[s_lo:s_lo + S_PC, o_lo:o_lo + O_PC]       # [4, 512]
        bias[:, 0:nso] = eb.reshape(S_PC, OT, P).transpose(2, 0, 1).reshape(P, -1)
        bs = bias_sigma[o_lo:o_lo + O_PC].reshape(OT, P).T   # [p, ot]
        bm = bias_mu[o_lo:o_lo + O_PC].reshape(OT, P).T
        bias[:, nso:2 * nso] = np.tile(bs, (1, S_PC))
        bias[:, 2 * nso:3 * nso] = np.tile(bm, (1, S_PC))

        in_maps.append({"img": img, "mu8": mu8, "bias": bias})
    return in_maps


def kernel(x, weight_mu, weight_sigma, bias_mu, bias_sigma, eps_w, eps_b):
    global _last_in_maps
    x = np.ascontiguousarray(x, dtype=np.float32)
    weight_mu = np.ascontiguousarray(weight_mu, dtype=np.float32)
    weight_sigma = np.ascontiguousarray(weight_sigma, dtype=np.float32)
    bias_mu = np.ascontiguousarray(bias_mu, dtype=np.float32)
    bias_sigma = np.ascontiguousarray(bias_sigma, dtype=np.float32)
    eps_w = np.ascontiguousarray(eps_w, dtype=np.float32)
    eps_b = np.ascontiguousarray(eps_b, dtype=np.float32)

    nc = _get_program()
    in_maps = _pack_inputs(x, weight_mu, weight_sigma, bias_mu, bias_sigma,
                           eps_w, eps_b)
    _last_in_maps = in_maps
    res = run_bass_kernel_spmd(nc, in_maps, core_ids=list(range(N_CORES)))

    out = np.empty((S, B, OUT), dtype=np.float32)
    for c in range(N_CORES):
        sg, og = divmod(c, OUT_WAYS)
        oT = np.asarray(res.results[c]["out"], dtype=np.float32)  # [4, 512, 256]
        out[sg * S_PC:(sg + 1) * S_PC, :, og * O_PC:(og + 1) * O_PC] = \
            oT.transpose(0, 2, 1)
    return out
